# revision 1
# baseline (speedup 1.0000x reference)
"""CGC (Customized Gate Control) MoE kernel for Trainium2, 8 NeuronCores.

Problem: 3 inputs x_{shared,task1,task2} [4096, 1024]; three expert groups
(sh/t1/t2) of 4 experts each; expert = fc2(relu(fc1(x))) with
fc1: 1024->2048, fc2: 2048->512; three softmax gates; outputs
(out_sh, out1, out2) each [4096, 512] as gate-weighted sums of expert
outputs.

Sharding: data-parallel over batch across 8 cores (512 rows/core), all
weights replicated. No collectives.

Per-core dataflow (batch tile b=512, partition tiles of 128):
  - x [512,1024] -> PE-transpose -> xT [1024, 512] in SBUF (fp32r)
  - gates: logits = xT.T @ wg + bg (PE) -> softmax (DVE+ACT), batch-major
  - per expert e: hT[ht] = relu(W1[:,ht].T @ xT + b1) (PE + DVE), fp32r
                  o[bt] += hT[:,bt].T @ W2[ht] over ht (PE, PSUM accum)
                  o[bt] += ones.T @ b2 (PE)
                  acc[head][bt] (+)= g[head][:,e] * o[bt] (DVE)
  - store acc -> outputs.

All matmuls run in float32r (TF32-like: full PE rate at N>=256, ~1.4e-4
relative error measured on HW vs fp32).
"""
import sys
from contextlib import nullcontext

if "/opt/trn_rl_repo" not in sys.path:
    sys.path.insert(0, "/opt/trn_rl_repo")

import numpy as np

import concourse.bass as bass
import concourse.mybir as mybir
from concourse import bacc
from concourse.tile import TileContext
from concourse.masks import make_identity

B, I, H, O = 4096, 1024, 2048, 512
E = 4                      # experts per group
N_CORES = 8
BL = B // N_CORES          # 512 rows per core
BT = BL // 128             # 4 batch tiles
IT = I // 128              # 8 input tiles
HT = H // 128              # 16 hidden tiles

F32 = mybir.dt.float32
F32R = mybir.dt.float32r

GROUPS = ("t1", "t2", "sh")
GATE_W = {"sh": 2 * E + E, "t1": E + E, "t2": E + E}  # 12, 8, 8


# (group, e) -> list of (head, gate_name, gate_col)
def _contribs(grp, e):
    if grp == "t1":
        return [("o1", "t1", e), ("osh", "sh", e)]
    if grp == "t2":
        return [("o2", "t2", e), ("osh", "sh", E + e)]
    return [("o1", "t1", E + e), ("o2", "t2", E + e), ("osh", "sh", 2 * E + e)]


def build_nc(loop_reps=None, mode="full"):
    """Build the per-core kernel. loop_reps wraps the whole body in a
    hardware For_i loop; mode selects "full", "dma" (loads only) or
    "compute" (no weight loads) - both diagnostics-only."""
    nc = bacc.Bacc(None)

    # ---- DRAM parameters ----------------------------------------------
    xs = {g: nc.declare_dram_parameter(f"x_{g}", [BL, I], F32, isOutput=False)
          for g in GROUPS}
    w1 = {g: nc.declare_dram_parameter(f"w1_{g}", [E, I, H], F32R, isOutput=False)
          for g in GROUPS}
    b1 = {g: nc.declare_dram_parameter(f"b1_{g}", [E, H], F32, isOutput=False)
          for g in GROUPS}
    w2 = {g: nc.declare_dram_parameter(f"w2_{g}", [E, H, O], F32R, isOutput=False)
          for g in GROUPS}
    b2 = {g: nc.declare_dram_parameter(f"b2_{g}", [E, O], F32R, isOutput=False)
          for g in GROUPS}
    wg = {g: nc.declare_dram_parameter(f"wg_{g}", [I, GATE_W[g]], F32R, isOutput=False)
          for g in GROUPS}
    bg = {g: nc.declare_dram_parameter(f"bg_{g}", [GATE_W[g]], F32R, isOutput=False)
          for g in GROUPS}
    outs = {h: nc.declare_dram_parameter(h, [BL, O], F32, isOutput=True)
            for h in ("osh", "o1", "o2")}

    with TileContext(nc) as tc:
        with tc.tile_pool(name="persist", bufs=1) as pp, \
             tc.tile_pool(name="work", bufs=1) as pw, \
             tc.tile_pool(name="ps", bufs=1, space="PSUM") as ps:
            # persistent SBUF: xT per group, gates, accumulators, consts
            xT = {g: pp.tile([128, IT, BL], F32R, name=f"xT_{g}") for g in GROUPS}
            gsb = {g: pp.tile([128, BT, GATE_W[g]], F32, name=f"g_{g}")
                   for g in GROUPS}
            acc = {h: pp.tile([128, BT, O], F32, name=f"acc_{h}")
                   for h in ("osh", "o1", "o2")}
            ident = pp.tile([128, 128], F32, name="ident")
            make_identity(nc, ident[:, :])
            ones_f = pp.tile([1, 128], F32, name="ones_f")
            nc.gpsimd.memset(ones_f[:, :], 1.0)
            ones = pp.tile([1, 128], F32R, name="ones")
            nc.vector.tensor_copy(ones[:, :], ones_f[:, :])

            gate_w = {}
            loop_cm = tc.For_i(0, loop_reps, 1) if loop_reps else nullcontext()
            with loop_cm:
                # ---- Phase A: all transposes first, then gates ---------
                for g in GROUPS:
                    for bt in range(BT):
                        x_sb = pw.tile([128, I], F32, tag="x_stage", bufs=3,
                                       name=f"xs_{g}_{bt}")
                        nc.sync.dma_start(x_sb[:, :],
                                          xs[g][bt * 128:(bt + 1) * 128, :])
                        for it in range(IT):
                            pt = ps.tile([128, 128], F32, tag="ph", bufs=4,
                                         name=f"pt_{g}_{bt}_{it}")
                            nc.tensor.transpose(
                                pt[:, :], x_sb[:, it * 128:(it + 1) * 128],
                                ident[:, :])
                            nc.vector.tensor_copy(
                                xT[g][:, it, bt * 128:(bt + 1) * 128], pt[:, :])
                    wg_sb = pw.tile([128, IT, GATE_W[g]], F32R, tag=f"wg{g}",
                                    bufs=1, name=f"wg_{g}_sb")
                    nc.sync.dma_start(
                        wg_sb[:, :, :],
                        wg[g].rearrange("(it p) e -> p it e", p=128))
                    bg_sb = pw.tile([1, GATE_W[g]], F32R, tag=f"bg{g}", bufs=1,
                                    name=f"bg_{g}_sb")
                    nc.sync.dma_start(bg_sb[:, :], bg[g][None, :])
                    gate_w[g] = (wg_sb, bg_sb)
                for g in GROUPS:
                    wg_sb, bg_sb = gate_w[g]
                    for bt in range(BT):
                        gps = ps.tile([128, GATE_W[g]], F32, tag="ph", bufs=4,
                                      name=f"gps_{g}_{bt}")
                        for it in range(IT):
                            nc.tensor.matmul(
                                gps[:, :],
                                xT[g][:, it, bt * 128:(bt + 1) * 128],
                                wg_sb[:, it, :],
                                start=(it == 0), stop=False)
                        nc.tensor.matmul(gps[:, :], ones[:, :], bg_sb[:, :],
                                         start=False, stop=True)
                        # softmax over free dim
                        mx = pw.tile([128, 1], F32, tag="mx", bufs=2,
                                     name=f"mx_{g}_{bt}")
                        nc.vector.reduce_max(mx[:, :], gps[:, :],
                                             axis=mybir.AxisListType.X)
                        nmx = pw.tile([128, 1], F32, tag="nmx", bufs=2,
                                      name=f"nmx_{g}_{bt}")
                        nc.vector.tensor_scalar_mul(nmx[:, :], mx[:, :], -1.0)
                        ex = pw.tile([128, GATE_W[g]], F32, tag="ex", bufs=2,
                                     name=f"ex_{g}_{bt}")
                        nc.scalar.activation(ex[:, :], gps[:, :],
                                             mybir.ActivationFunctionType.Exp,
                                             bias=nmx[:, :], scale=1.0)
                        sm = pw.tile([128, 1], F32, tag="sm", bufs=2,
                                     name=f"sm_{g}_{bt}")
                        nc.vector.reduce_sum(sm[:, :], ex[:, :],
                                             axis=mybir.AxisListType.X)
                        rs = pw.tile([128, 1], F32, tag="rs", bufs=2,
                                     name=f"rs_{g}_{bt}")
                        nc.vector.reciprocal(rs[:, :], sm[:, :])
                        nc.vector.tensor_scalar_mul(gsb[g][:, bt, :], ex[:, :],
                                                    rs[:, :])

                # ---- Phase B: experts, fc2 software-pipelined by one ----
                # PE queue is in-order; emitting mm2(ht) right after mm1(ht)
                # would stall PE on the relu(ht) dependency. Instead mm2(ht)
                # is emitted after mm1(ht+1), so the relu latency hides under
                # the next fc1 block.
                first_seen = set()
                HTG = 512 // 128  # ht-tiles per W1/W2 column block
                expert_bias = {}
                expert_psum = {}

                def emit_mm2(g, e, ht, hT, w2t, ht4):
                    if ht == 0:
                        expert_psum[(g, e)] = [
                            ps.tile([128, O], F32, tag=f"po{bt}", bufs=1,
                                    name=f"po_{g}_{e}_{bt}")
                            for bt in range(BT)]
                    psum_o = expert_psum[(g, e)]
                    for bt in range(BT):
                        nc.tensor.matmul(
                            psum_o[bt][:, :],
                            hT[:, bt * 128:(bt + 1) * 128],
                            w2t[:, ht4, :],
                            start=(ht == 0), stop=False)
                    if ht != HT - 1:
                        return
                    # expert tail: fc2 bias, PSUM drain, gated accumulation
                    b2_sb = expert_bias[(g, e)][1]
                    for bt in range(BT):
                        nc.tensor.matmul(psum_o[bt][:, :], ones[:, :],
                                         b2_sb[:, :], start=False, stop=True)
                    for bt in range(BT):
                        o_sb = pw.tile([128, O], F32, tag="o_sb", bufs=4,
                                       name=f"osb_{g}{e}_{bt}")
                        nc.scalar.copy(o_sb[:, :], psum_o[bt][:, :])
                        for head, gate, col in _contribs(g, e):
                            gcol = gsb[gate][:, bt, col:col + 1]
                            if (head, bt) not in first_seen:
                                nc.vector.tensor_scalar_mul(
                                    acc[head][:, bt, :], o_sb[:, :], gcol)
                                first_seen.add((head, bt))
                            else:
                                nc.vector.scalar_tensor_tensor(
                                    acc[head][:, bt, :], o_sb[:, :],
                                    gcol, acc[head][:, bt, :],
                                    op0=mybir.AluOpType.mult,
                                    op1=mybir.AluOpType.add)

                pending = []
                SKEW = 2
                step = 0
                for g in GROUPS:
                    for e in range(E):
                        b1_sb = pw.tile([128, HT], F32, tag="b1", bufs=2,
                                        name=f"b1_{g}{e}")
                        nc.sync.dma_start(
                            b1_sb[:, :],
                            b1[g][e].rearrange("(ht p) -> p ht", p=128))
                        b2_sb = pw.tile([1, O], F32R, tag="b2", bufs=2,
                                        name=f"b2_{g}{e}")
                        nc.sync.dma_start(b2_sb[:, :], b2[g][e][None, :])
                        expert_bias[(g, e)] = (b1_sb, b2_sb)
                        for ht in range(HT):
                            htg, ht4 = divmod(ht, HTG)
                            if ht4 == 0:
                                # W1 column block [1024, 512] -> 2KB DMA beats
                                w1t = pw.tile([128, IT, 512], F32R, tag="w1",
                                              bufs=3, name=f"w1_{g}{e}_{htg}")
                                if mode != "compute":
                                    nc.sync.dma_start(
                                        w1t[:, :, :],
                                        w1[g][e, :, htg * 512:(htg + 1) * 512]
                                        .rearrange("(it p) h -> p it h", p=128))
                                else:
                                    nc.sync.dma_start(
                                        w1t[:, 0, 0:1],
                                        w1[g][e, 0:128, htg * 512:htg * 512 + 1]
                                        .rearrange("p h -> p h"))
                                # W2 row block [512, 512] -> 2KB DMA beats
                                w2t = pw.tile([128, HTG, O], F32R, tag="w2",
                                              bufs=3, name=f"w2_{g}{e}_{htg}")
                                if mode != "compute":
                                    nc.sync.dma_start(
                                        w2t[:, :, :],
                                        w2[g][e, htg * 512:(htg + 1) * 512, :]
                                        .rearrange("(hh p) o -> p hh o", p=128))
                                else:
                                    nc.sync.dma_start(
                                        w2t[:, 0, 0:1],
                                        w2[g][e, htg * 512:htg * 512 + 128, 0:1])

                            if mode == "dma":
                                continue
                            ph = ps.tile([128, BL], F32, tag="ph", bufs=4,
                                         name=f"ph_{g}{e}_{ht}")
                            for it in range(IT):
                                nc.tensor.matmul(
                                    ph[:, :],
                                    w1t[:, it, ht4 * 128:(ht4 + 1) * 128],
                                    xT[g][:, it, :],
                                    start=(it == 0),
                                    stop=(it == IT - 1))
                            hT = pw.tile([128, BL], F32R, tag="hT", bufs=6,
                                         name=f"hT_{g}{e}_{ht}")
                            # relu(ph + b1) -> fp32r; alternate DVE/ACT to
                            # split the epilogue load across both engines
                            if step % 2 == 0:
                                nc.vector.tensor_scalar(
                                    hT[:, :], ph[:, :],
                                    b1_sb[:, ht:ht + 1], 0.0,
                                    op0=mybir.AluOpType.add,
                                    op1=mybir.AluOpType.max)
                            else:
                                nc.scalar.activation(
                                    hT[:, :], ph[:, :],
                                    mybir.ActivationFunctionType.Relu,
                                    bias=b1_sb[:, ht:ht + 1], scale=1.0)
                            pending.append((g, e, ht, hT, w2t, ht4))
                            if len(pending) > SKEW:
                                emit_mm2(*pending.pop(0))
                            step += 1
                while pending:
                    emit_mm2(*pending.pop(0))

                # ---- store outputs -----------------------------------
                for h in (() if mode == "dma" else ("osh", "o1", "o2")):
                    for bt in range(BT):
                        nc.sync.dma_start(outs[h][bt * 128:(bt + 1) * 128, :],
                                          acc[h][:, bt, :])

    nc.finalize()
    return nc


_NC_CACHE = None


def _get_nc():
    global _NC_CACHE
    if _NC_CACHE is None:
        _NC_CACHE = build_nc()
    return _NC_CACHE


def kernel(**inputs) -> tuple:
    from concourse.bass_utils import run_bass_kernel_spmd

    nc = _get_nc()
    np_in = {k: np.ascontiguousarray(np.asarray(v, dtype=np.float32))
             for k, v in inputs.items()}
    in_maps = []
    for c in range(N_CORES):
        sl = slice(c * BL, (c + 1) * BL)
        m = {
            "x_sh": np.ascontiguousarray(np_in["x_shared"][sl]),
            "x_t1": np.ascontiguousarray(np_in["x_task1"][sl]),
            "x_t2": np.ascontiguousarray(np_in["x_task2"][sl]),
        }
        for g in GROUPS:
            for pfx in ("w1", "b1", "w2", "b2", "wg", "bg"):
                m[f"{pfx}_{g}"] = np_in[f"{pfx}_{g}"]
        in_maps.append(m)

    # rare transient NRT_EXEC_UNIT_UNRECOVERABLE crashes have been observed
    # on this fabric; retry a couple of times before giving up
    last_err = None
    for attempt in range(3):
        try:
            r = run_bass_kernel_spmd(nc, in_maps, list(range(N_CORES)))
            break
        except Exception as ex:  # noqa: BLE001
            last_err = ex
            import time as _time
            _time.sleep(5 * (attempt + 1))
    else:
        raise last_err
    out_sh = np.concatenate([r.results[c]["osh"] for c in range(N_CORES)], axis=0)
    out1 = np.concatenate([r.results[c]["o1"] for c in range(N_CORES)], axis=0)
    out2 = np.concatenate([r.results[c]["o2"] for c in range(N_CORES)], axis=0)
    return (out_sh, out1, out2)



# revision 4
# speedup vs baseline: 1.4608x; 1.4608x over previous
"""CGC (Customized Gate Control) MoE kernel for Trainium2, 8 NeuronCores.

Problem: 3 inputs x_{shared,task1,task2} [4096, 1024]; three expert groups
(sh/t1/t2) of 4 experts each; expert = fc2(relu(fc1(x))) with
fc1: 1024->2048, fc2: 2048->512; three softmax gates; outputs
(out_sh, out1, out2) each [4096, 512] as gate-weighted sums of expert
outputs.

Sharding: data-parallel over batch across 8 cores (512 rows/core), all
weights replicated. No collectives.

v2 changes vs v1 (522us):
  - weights/x/wg cast to bf16 on the host: halves HBM weight streaming
    (151MB -> 76MB per core-iteration), which was ~94% of a DMA queue;
    matmul throughput unchanged (PE streams 1 col/cycle for bf16 and
    fp32r alike), accuracy ~1e-3 rel (gate is 2e-2).
  - fc2 bias: instead of 48 ones@b2 PE matmuls (one per expert x bt,
    24.6k PE cycles), compute the gate-weighted bias term per head once:
    bias_head = g_head @ B2_head via 12 small matmuls with the transposed
    gate matrix as stationary (7.7k cycles incl. gate transposes), and
    initialize the output accumulator with it.
  - PSUM->SBUF copies split between DVE and ACT engines to kill the
    phase-A stalls where PE waited on transposed-x copies.

Per-core dataflow (batch tile b=512, partition tiles of 128):
  - x [512,1024] bf16 -> PE-transpose -> xT [1024, 512] in SBUF
  - gates: logits = xT.T @ wg + bg (PE) -> softmax (DVE+ACT), batch-major
  - gate transpose gT (PE), bias_head = gT.T @ B2 (PE) -> acc init
  - per expert e: hT[ht] = relu(W1[:,ht].T @ xT + b1) (PE + DVE/ACT), bf16
                  o[bt] += hT[:,bt].T @ W2[ht] over ht (PE, PSUM accum)
                  acc[head][bt] += g[head][:,e] * o[bt] (DVE)
  - store acc -> outputs.
"""
import sys
from contextlib import nullcontext

if "/opt/trn_rl_repo" not in sys.path:
    sys.path.insert(0, "/opt/trn_rl_repo")

import numpy as np
import ml_dtypes

import concourse.bass as bass
import concourse.mybir as mybir
from concourse import bacc
from concourse.tile import TileContext
from concourse.masks import make_identity

B, I, H, O = 4096, 1024, 2048, 512
E = 4                      # experts per group
N_CORES = 8
BL = B // N_CORES          # 512 rows per core
BT = BL // 128             # 4 batch tiles
IT = I // 128              # 8 input tiles
HT = H // 128              # 16 hidden tiles

F32 = mybir.dt.float32
F32R = mybir.dt.float32r
BF16 = mybir.dt.bfloat16

GROUPS = ("t1", "t2", "sh")
GATE_W = {"sh": 2 * E + E, "t1": E + E, "t2": E + E}  # 12, 8, 8
# head -> (gate group, [(expert group, base col), ...])
HEADS = {
    "o1": ("t1", [("t1", 0), ("sh", E)]),
    "o2": ("t2", [("t2", 0), ("sh", E)]),
    "osh": ("sh", [("t1", 0), ("t2", E), ("sh", 2 * E)]),
}


# (group, e) -> list of (head, gate_name, gate_col)
def _contribs(grp, e):
    if grp == "t1":
        return [("o1", "t1", e), ("osh", "sh", e)]
    if grp == "t2":
        return [("o2", "t2", e), ("osh", "sh", E + e)]
    return [("o1", "t1", E + e), ("o2", "t2", E + e), ("osh", "sh", 2 * E + e)]


def build_nc(loop_reps=None, mode="full"):
    """Build the per-core kernel. loop_reps wraps the whole body in a
    hardware For_i loop; mode selects "full", "dma" (loads only) or
    "compute" (no weight loads) - both diagnostics-only."""
    nc = bacc.Bacc(None)

    # ---- DRAM parameters ----------------------------------------------
    xs = {g: nc.declare_dram_parameter(f"x_{g}", [BL, I], BF16, isOutput=False)
          for g in GROUPS}
    w1 = {g: nc.declare_dram_parameter(f"w1_{g}", [E, I, H], BF16, isOutput=False)
          for g in GROUPS}
    b1 = {g: nc.declare_dram_parameter(f"b1_{g}", [E, H], F32, isOutput=False)
          for g in GROUPS}
    w2 = {g: nc.declare_dram_parameter(f"w2_{g}", [E, H, O], BF16, isOutput=False)
          for g in GROUPS}
    b2 = {g: nc.declare_dram_parameter(f"b2_{g}", [E, O], F32R, isOutput=False)
          for g in GROUPS}
    wg = {g: nc.declare_dram_parameter(f"wg_{g}", [I, GATE_W[g]], BF16, isOutput=False)
          for g in GROUPS}
    bg = {g: nc.declare_dram_parameter(f"bg_{g}", [GATE_W[g]], BF16, isOutput=False)
          for g in GROUPS}
    outs = {h: nc.declare_dram_parameter(h, [BL, O], F32, isOutput=True)
            for h in ("osh", "o1", "o2")}

    with TileContext(nc) as tc:
        with tc.tile_pool(name="persist", bufs=1) as pp, \
             tc.tile_pool(name="work", bufs=1) as pw, \
             tc.tile_pool(name="ps", bufs=1, space="PSUM") as ps:
            # persistent SBUF: xT per group, gates, accumulators, consts
            xT = {g: pp.tile([128, IT, BL], BF16, name=f"xT_{g}") for g in GROUPS}
            gsb = {g: pp.tile([128, BT, GATE_W[g]], F32, name=f"g_{g}")
                   for g in GROUPS}
            acc = {h: pp.tile([128, BT, O], F32, name=f"acc_{h}")
                   for h in ("osh", "o1", "o2")}
            ident = pp.tile([128, 128], F32, name="ident")
            make_identity(nc, ident[:, :])
            ident_bf = pp.tile([128, 128], BF16, name="ident_bf")
            nc.vector.tensor_copy(ident_bf[:, :], ident[:, :])
            ones_f = pp.tile([1, 128], F32, name="ones_f")
            nc.gpsimd.memset(ones_f[:, :], 1.0)
            ones_bf = pp.tile([1, 128], BF16, name="ones_bf")
            nc.vector.tensor_copy(ones_bf[:, :], ones_f[:, :])

            gate_w = {}
            loop_cm = tc.For_i(0, loop_reps, 1) if loop_reps else nullcontext()
            with loop_cm:
                # ---- Phase A: transposes, gates, head-bias -------------
                ncp = 0

                def cp(dst, src):  # alternate DVE / ACT for PSUM drains
                    nonlocal ncp
                    if ncp % 2 == 0:
                        nc.vector.tensor_copy(dst, src)
                    else:
                        nc.scalar.copy(dst, src)
                    ncp += 1
                for g in GROUPS:
                    for bt in range(BT):
                        x_sb = pw.tile([128, I], BF16, tag="x_stage", bufs=3,
                                       name=f"xs_{g}_{bt}")
                        nc.sync.dma_start(x_sb[:, :],
                                          xs[g][bt * 128:(bt + 1) * 128, :])
                        for it in range(IT):
                            pt = ps.tile([128, 128], BF16, tag="ph", bufs=4,
                                         name=f"pt_{g}_{bt}_{it}")
                            nc.tensor.transpose(
                                pt[:, :], x_sb[:, it * 128:(it + 1) * 128],
                                ident_bf[:, :])
                            cp(xT[g][:, it, bt * 128:(bt + 1) * 128],
                               pt[:, :])
                    wg_sb = pw.tile([128, IT, GATE_W[g]], BF16, tag=f"wg{g}",
                                    bufs=1, name=f"wg_{g}_sb")
                    nc.sync.dma_start(
                        wg_sb[:, :, :],
                        wg[g].rearrange("(it p) e -> p it e", p=128))
                    bg_sb = pw.tile([1, GATE_W[g]], BF16, tag=f"bg{g}", bufs=1,
                                    name=f"bg_{g}_sb")
                    nc.sync.dma_start(bg_sb[:, :], bg[g][None, :])
                    gate_w[g] = (wg_sb, bg_sb)
                # stacked per-head fc2 biases [E_head, O] (f32r)
                b2_sb = {}
                for h, (gate, parts) in HEADS.items():
                    t = pw.tile([GATE_W[gate], O], F32R, tag=f"b2{h}", bufs=1,
                                name=f"b2_{h}_sb")
                    for pg, base in parts:
                        nc.sync.dma_start(t[base:base + E, :], b2[pg][:, :])
                    b2_sb[h] = t
                # gates: logits -> softmax (batch-major) -> transposed gT
                gT = {g: pw.tile([GATE_W[g], BT, 128], F32R, tag=f"gT{g}",
                                 bufs=1, name=f"gT_{g}")
                      for g in GROUPS}
                for g in GROUPS:
                    wg_sb, bg_sb = gate_w[g]
                    for bt in range(BT):
                        gps = ps.tile([128, GATE_W[g]], F32, tag="ph", bufs=4,
                                      name=f"gps_{g}_{bt}")
                        for it in range(IT):
                            nc.tensor.matmul(
                                gps[:, :],
                                xT[g][:, it, bt * 128:(bt + 1) * 128],
                                wg_sb[:, it, :],
                                start=(it == 0), stop=False)
                        nc.tensor.matmul(gps[:, :], ones_bf[:, :], bg_sb[:, :],
                                         start=False, stop=True)
                        # softmax over free dim
                        mx = pw.tile([128, 1], F32, tag="mx", bufs=2,
                                     name=f"mx_{g}_{bt}")
                        nc.vector.reduce_max(mx[:, :], gps[:, :],
                                             axis=mybir.AxisListType.X)
                        nmx = pw.tile([128, 1], F32, tag="nmx", bufs=2,
                                      name=f"nmx_{g}_{bt}")
                        nc.vector.tensor_scalar_mul(nmx[:, :], mx[:, :], -1.0)
                        ex = pw.tile([128, GATE_W[g]], F32, tag="ex", bufs=2,
                                     name=f"ex_{g}_{bt}")
                        nc.scalar.activation(ex[:, :], gps[:, :],
                                             mybir.ActivationFunctionType.Exp,
                                             bias=nmx[:, :], scale=1.0)
                        sm = pw.tile([128, 1], F32, tag="sm", bufs=2,
                                     name=f"sm_{g}_{bt}")
                        nc.vector.reduce_sum(sm[:, :], ex[:, :],
                                             axis=mybir.AxisListType.X)
                        rs = pw.tile([128, 1], F32, tag="rs", bufs=2,
                                     name=f"rs_{g}_{bt}")
                        nc.vector.reciprocal(rs[:, :], sm[:, :])
                        nc.vector.tensor_scalar_mul(gsb[g][:, bt, :], ex[:, :],
                                                    rs[:, :])
                        # transposed gates for the head-bias matmul
                        gtp = ps.tile([GATE_W[g], 128], F32, tag="ph", bufs=4,
                                      name=f"gtp_{g}_{bt}")
                        nc.tensor.transpose(gtp[:, :], gsb[g][:, bt, :],
                                            ident[:, :])
                        cp(gT[g][:, bt, :], gtp[:, :])
                # head-bias: acc[h][bt] = g_head[bt] @ B2_head
                for h, (gate, _) in HEADS.items():
                    for bt in range(BT):
                        pb = ps.tile([128, O], F32, tag=f"po{bt}", bufs=1,
                                     name=f"pb_{h}_{bt}")
                        nc.tensor.matmul(pb[:, :], gT[gate][:, bt, :],
                                         b2_sb[h][:, :], start=True, stop=True)
                        cp(acc[h][:, bt, :], pb[:, :])

                # ---- Phase B: experts, fc2 software-pipelined by one ----
                # PE queue is in-order; emitting mm2(ht) right after mm1(ht)
                # would stall PE on the relu(ht) dependency. Instead mm2(ht)
                # is emitted after mm1(ht+1), so the relu latency hides under
                # the next fc1 block.
                HTG = 512 // 128  # ht-tiles per W1/W2 column block
                expert_bias = {}
                expert_psum = {}

                def emit_mm2(g, e, ht, hT, w2t, ht4):
                    if ht == 0:
                        expert_psum[(g, e)] = [
                            ps.tile([128, O], F32, tag=f"po{bt}", bufs=1,
                                    name=f"po_{g}_{e}_{bt}")
                            for bt in range(BT)]
                    psum_o = expert_psum[(g, e)]
                    for bt in range(BT):
                        nc.tensor.matmul(
                            psum_o[bt][:, :],
                            hT[:, bt * 128:(bt + 1) * 128],
                            w2t[:, ht4, :],
                            start=(ht == 0), stop=(ht == HT - 1))
                    if ht != HT - 1:
                        return
                    # expert tail: PSUM drain, gated accumulation
                    for bt in range(BT):
                        o_sb = pw.tile([128, O], F32, tag="o_sb", bufs=4,
                                       name=f"osb_{g}{e}_{bt}")
                        nc.scalar.copy(o_sb[:, :], psum_o[bt][:, :])
                        for head, gate, col in _contribs(g, e):
                            gcol = gsb[gate][:, bt, col:col + 1]
                            nc.vector.scalar_tensor_tensor(
                                acc[head][:, bt, :], o_sb[:, :],
                                gcol, acc[head][:, bt, :],
                                op0=mybir.AluOpType.mult,
                                op1=mybir.AluOpType.add)

                pending = []
                SKEW = 2
                step = 0
                for g in GROUPS:
                    for e in range(E):
                        b1_sb = pw.tile([128, HT], F32, tag="b1", bufs=2,
                                        name=f"b1_{g}{e}")
                        nc.sync.dma_start(
                            b1_sb[:, :],
                            b1[g][e].rearrange("(ht p) -> p ht", p=128))
                        expert_bias[(g, e)] = b1_sb
                        for ht in range(HT):
                            htg, ht4 = divmod(ht, HTG)
                            if ht4 == 0:
                                # W1 column block [1024, 512] -> 1KB DMA beats
                                w1t = pw.tile([128, IT, 512], BF16, tag="w1",
                                              bufs=3, name=f"w1_{g}{e}_{htg}")
                                if mode != "compute":
                                    nc.sync.dma_start(
                                        w1t[:, :, :],
                                        w1[g][e, :, htg * 512:(htg + 1) * 512]
                                        .rearrange("(it p) h -> p it h", p=128))
                                else:
                                    nc.sync.dma_start(
                                        w1t[:, 0, 0:1],
                                        w1[g][e, 0:128, htg * 512:htg * 512 + 1]
                                        .rearrange("p h -> p h"))
                                # W2 row block [512, 512] -> 1KB DMA beats
                                w2t = pw.tile([128, HTG, O], BF16, tag="w2",
                                              bufs=3, name=f"w2_{g}{e}_{htg}")
                                if mode != "compute":
                                    nc.sync.dma_start(
                                        w2t[:, :, :],
                                        w2[g][e, htg * 512:(htg + 1) * 512, :]
                                        .rearrange("(hh p) o -> p hh o", p=128))
                                else:
                                    nc.sync.dma_start(
                                        w2t[:, 0, 0:1],
                                        w2[g][e, htg * 512:htg * 512 + 128, 0:1])

                            if mode == "dma":
                                continue
                            ph = ps.tile([128, BL], F32, tag="ph", bufs=4,
                                         name=f"ph_{g}{e}_{ht}")
                            for it in range(IT):
                                nc.tensor.matmul(
                                    ph[:, :],
                                    w1t[:, it, ht4 * 128:(ht4 + 1) * 128],
                                    xT[g][:, it, :],
                                    start=(it == 0),
                                    stop=(it == IT - 1))
                            hT = pw.tile([128, BL], BF16, tag="hT", bufs=6,
                                         name=f"hT_{g}{e}_{ht}")
                            # relu(ph + b1) -> bf16; alternate DVE/ACT to
                            # split the epilogue load across both engines
                            if step % 2 == 0:
                                nc.vector.tensor_scalar(
                                    hT[:, :], ph[:, :],
                                    b1_sb[:, ht:ht + 1], 0.0,
                                    op0=mybir.AluOpType.add,
                                    op1=mybir.AluOpType.max)
                            else:
                                nc.scalar.activation(
                                    hT[:, :], ph[:, :],
                                    mybir.ActivationFunctionType.Relu,
                                    bias=b1_sb[:, ht:ht + 1], scale=1.0)
                            pending.append((g, e, ht, hT, w2t, ht4))
                            if len(pending) > SKEW:
                                emit_mm2(*pending.pop(0))
                            step += 1
                while pending:
                    emit_mm2(*pending.pop(0))

                # ---- store outputs -----------------------------------
                for h in (() if mode == "dma" else ("osh", "o1", "o2")):
                    for bt in range(BT):
                        nc.sync.dma_start(outs[h][bt * 128:(bt + 1) * 128, :],
                                          acc[h][:, bt, :])

    nc.finalize()
    return nc


def make_in_maps(np_in):
    """Host-side marshalling: slice the batch per core and cast the big
    operands (x, W1, W2, wg, bg) to bf16; biases stay fp32."""
    bf = ml_dtypes.bfloat16
    wcast = {}
    for g in GROUPS:
        wcast[f"w1_{g}"] = np.ascontiguousarray(np_in[f"w1_{g}"].astype(bf))
        wcast[f"w2_{g}"] = np.ascontiguousarray(np_in[f"w2_{g}"].astype(bf))
        wcast[f"wg_{g}"] = np.ascontiguousarray(np_in[f"wg_{g}"].astype(bf))
        wcast[f"bg_{g}"] = np.ascontiguousarray(np_in[f"bg_{g}"].astype(bf))
        wcast[f"b1_{g}"] = np.ascontiguousarray(np_in[f"b1_{g}"].astype(np.float32))
        wcast[f"b2_{g}"] = np.ascontiguousarray(np_in[f"b2_{g}"].astype(np.float32))
    in_maps = []
    for c in range(N_CORES):
        sl = slice(c * BL, (c + 1) * BL)
        m = {
            "x_sh": np.ascontiguousarray(np_in["x_shared"][sl].astype(bf)),
            "x_t1": np.ascontiguousarray(np_in["x_task1"][sl].astype(bf)),
            "x_t2": np.ascontiguousarray(np_in["x_task2"][sl].astype(bf)),
        }
        m.update(wcast)
        in_maps.append(m)
    return in_maps


_NC_CACHE = None


def _get_nc():
    global _NC_CACHE
    if _NC_CACHE is None:
        _NC_CACHE = build_nc()
    return _NC_CACHE


def kernel(**inputs) -> tuple:
    from concourse.bass_utils import run_bass_kernel_spmd

    nc = _get_nc()
    np_in = {k: np.asarray(v) for k, v in inputs.items()}
    in_maps = make_in_maps(np_in)

    # rare transient NRT_EXEC_UNIT_UNRECOVERABLE crashes have been observed
    # on this fabric; retry a couple of times before giving up
    last_err = None
    for attempt in range(3):
        try:
            r = run_bass_kernel_spmd(nc, in_maps, list(range(N_CORES)))
            break
        except Exception as ex:  # noqa: BLE001
            last_err = ex
            import time as _time
            _time.sleep(5 * (attempt + 1))
    else:
        raise last_err
    out_sh = np.concatenate([r.results[c]["osh"] for c in range(N_CORES)], axis=0)
    out1 = np.concatenate([r.results[c]["o1"] for c in range(N_CORES)], axis=0)
    out2 = np.concatenate([r.results[c]["o2"] for c in range(N_CORES)], axis=0)
    return (out_sh, out1, out2)
